# revision 8
# baseline (speedup 1.0000x reference)
"""Causal attention (B=4, N=2048, D=1024) on 8 Trainium2 NeuronCores.

Sharding: core 2b+p owns batch b's token tiles {p, p+2, ..., p+14}
(parity-interleaved 128-row tiles).  Each core projects Q^T, K^T and V
only for its own 8 tiles (one x load feeds all three projections), then
the two cores of a batch exchange K^T/V halves in bf16 via two chunked
pairwise AllGathers that overlap with the remaining projection work.

Attention runs over keys in "gather layout" (all parity-0 tiles first,
then all parity-1 tiles) so the program is uniform across cores; the
causal masks are per-core input data applied to the tail tile of each
parity region.  Each slot i covers i+1 key tiles per region — an
exactly balanced causal split (the same trick as masking L=2(i+1)
contiguous tiles, reindexed).

Attention operands (Q^T, K^T, V, P) are bf16: full PE rate at any
moving width, half the exchange volume and SBUF footprint.  Projections
stay float32r (full PE rate at 512-wide).  Scores accumulate in f32
PSUM; softmax is single-pass (|scores|/32 bounded) with exp + row-sum
fused on the scalar engine.
"""
import sys

sys.path.insert(0, "/opt/trn_rl_repo")

from contextlib import ExitStack

import numpy as np

import concourse.bass as bass
import concourse.mybir as mybir
import concourse.tile as tile
from concourse import bacc
from concourse.bass_utils import run_bass_kernel_spmd
from concourse.masks import make_identity

B, N, D = 4, 2048, 1024
N_CORES = 8
N_OWN = 8            # own token tiles per core (q-slots == own k-tiles)
SCALE = 1.0 / 32.0   # 1/sqrt(D)
NEG = -1.0e9

F32 = mybir.dt.float32
F32R = mybir.dt.float32r
BF16 = mybir.dt.bfloat16

GROUPS = [[0, 1], [2, 3], [4, 5], [6, 7]]

_NC_CACHE = {}
TRACE = False
LAST_EXEC_NS = None


def _build_nc():
    nc = bacc.Bacc(None, target_bir_lowering=False, debug=False)

    # x for own tiles, d-major: [own_tile, p(d%128), dchunk, token]
    x_own = nc.declare_dram_parameter("x_own", [N_OWN, 128, 8, 128], F32R, isOutput=False)
    # weights host-rearranged: wq/wk [echunk, p(d%128), dchunk, ecol]; wv [eh, p, dchunk, ecol]
    wq = nc.declare_dram_parameter("wq", [8, 128, 8, 128], F32R, isOutput=False)
    wk = nc.declare_dram_parameter("wk", [8, 128, 8, 128], F32R, isOutput=False)
    wv = nc.declare_dram_parameter("wv", [2, 128, 8, 512], F32R, isOutput=False)
    mask_in = nc.declare_dram_parameter("mask", [128, 256], F32, isOutput=False)
    out_q = nc.declare_dram_parameter("out_q", [N_OWN, 128, D], F32, isOutput=True)

    with tile.TileContext(nc) as tc, ExitStack() as top:
        consts = top.enter_context(tc.tile_pool(name="consts", bufs=1))
        kt_pool = top.enter_context(tc.tile_pool(name="ktp", bufs=1))
        v_pool = top.enter_context(tc.tile_pool(name="vp", bufs=1))
        qt_pool = top.enter_context(tc.tile_pool(name="qtp", bufs=1))
        xt_pool = top.enter_context(tc.tile_pool(name="xtp", bufs=1))
        ccdram = top.enter_context(tc.tile_pool(name="ccd", bufs=1, space="DRAM"))

        ident_f = consts.tile([128, 128], F32)
        make_identity(nc, ident_f)
        ident = consts.tile([128, 128], BF16)
        nc.vector.tensor_copy(ident, ident_f)
        mask_sb = consts.tile([128, 256], F32)
        nc.sync.dma_start(out=mask_sb, in_=mask_in[:, :])

        KT = kt_pool.tile([128, 8, N], BF16)       # [p(e%128), echunk, gkey]
        Vt = v_pool.tile([128, 16, D], BF16)       # [p(tok%128), gtile, ecol]
        QT = qt_pool.tile([128, 8, 1024], BF16)    # [p(e%128), echunk, own-q]
        xT = xt_pool.tile([128, N_OWN, 8, 128], F32R)

        # CC bounce: rows 0-7 = K^T e-chunks [128, 512toks]; rows 8-15 = V
        # (own-tile tt, half eh) -> row 8+2tt+eh [128, 512 ecols]
        cin = [ccdram.tile([16, 128, 512], BF16, name=f"cin{h}") for h in range(2)]
        cout = [ccdram.tile([2, 16, 128, 512], BF16, name=f"cout{h}") for h in range(2)]

        for t in range(N_OWN):
            nc.gpsimd.dma_start(out=xT[:, t, :, :], in_=x_own[t][:, :, :])

        with ExitStack() as ph_p:
            wv_pool = ph_p.enter_context(tc.tile_pool(name="wvp", bufs=1))
            wk_pool = ph_p.enter_context(tc.tile_pool(name="wkp", bufs=1))
            stage = ph_p.enter_context(tc.tile_pool(name="stg", bufs=4))
            ps_mm = ph_p.enter_context(tc.tile_pool(name="psmm", bufs=8, space="PSUM"))

            # chunked wv load so the first V matmul only waits on ~2.5MB
            wv_sb = wv_pool.tile([128, 2, 8, 512], F32R)
            for c in range(8):
                for eh in range(2):
                    nc.scalar.dma_start(
                        out=wv_sb[:, eh, c, :], in_=wv[eh][:, c, :]
                    )
            wk_sb = wk_pool.tile([128, 8, 8, 128], F32R)
            for e in range(8):
                nc.scalar.dma_start(out=wk_sb[:, e, :, :], in_=wk[e][:, :, :])
            # prefetch first wq tiles early (no deps -> issue at t0)
            wq_sbs = {}
            for e in range(4):
                wq_sbs[e] = stage.tile([128, 8, 128], F32R, tag="wq",
                                       name=f"wq{e}", bufs=4)
                nc.scalar.dma_start(out=wq_sbs[e], in_=wq[e][:, :, :])

            for h in range(2):
                # V for own tiles 4h..4h+3
                for tt in range(4):
                    t = 4 * h + tt
                    for eh in range(2):
                        vps = ps_mm.tile([128, 512], F32, tag="mm", name=f"v{t}_{eh}")
                        for c in range(8):
                            nc.tensor.matmul(
                                vps, xT[:, t, c, :], wv_sb[:, eh, c, :],
                                start=(c == 0), stop=(c == 7),
                            )
                        vst = stage.tile([128, 512], BF16, tag="st", name=f"vs{t}_{eh}")
                        nc.scalar.activation(
                            vst, vps, mybir.ActivationFunctionType.Copy
                        )
                        nc.sync.dma_start(out=cin[h][8 + 2 * tt + eh], in_=vst)
                # K^T for own tiles 4h..4h+3 (512 token cols per matmul)
                for e in range(8):
                    kps = ps_mm.tile([128, 512], F32, tag="mm", name=f"k{h}_{e}")
                    for c in range(8):
                        nc.tensor.matmul(
                            kps, wk_sb[:, e, c, :], xT[:, 4 * h:4 * h + 4, c, :],
                            start=(c == 0), stop=(c == 7),
                        )
                    kst = stage.tile([128, 512], BF16, tag="st", name=f"ks{h}_{e}")
                    nc.vector.tensor_copy(kst, kps)
                    nc.sync.dma_start(out=cin[h][e], in_=kst)
                nc.gpsimd.collective_compute(
                    "AllGather",
                    mybir.AluOpType.bypass,
                    replica_groups=GROUPS,
                    ins=[cin[h][:, :, :].opt()],
                    outs=[cout[h][:, :, :, :].opt()],
                )

            # readback into gather layout (region r = parity-r core's half),
            # after both CC triggers so the rb waits never block a CC issue
            for h in range(2):
                for r in range(2):
                    nc.gpsimd.dma_start(
                        out=KT[:, :, r * 1024 + h * 512: r * 1024 + (h + 1) * 512],
                        in_=cout[h][r, 0:8].rearrange("e p c -> p e c"),
                    )
                    nc.gpsimd.dma_start(
                        out=Vt[:, r * 8 + 4 * h: r * 8 + 4 * h + 4, :].rearrange(
                            "p t (s c) -> p (t s) c", s=2
                        ),
                        in_=cout[h][r, 8:16].rearrange("e p c -> p e c"),
                    )

            # ---- Q^T projections (own tiles == slots), overlap the CC wall
            for e in range(8):
                if e not in wq_sbs:
                    wq_sbs[e] = stage.tile([128, 8, 128], F32R, tag="wq",
                                           name=f"wq{e}", bufs=4)
                    nc.scalar.dma_start(out=wq_sbs[e], in_=wq[e][:, :, :])
                for qg in range(2):
                    qps = ps_mm.tile([128, 512], F32, tag="mm", name=f"q{e}_{qg}")
                    for c in range(8):
                        nc.tensor.matmul(
                            qps, wq_sbs[e][:, c, :], xT[:, qg * 4:(qg + 1) * 4, c, :],
                            start=(c == 0), stop=(c == 7),
                        )
                    nc.scalar.activation(
                        QT[:, e, qg * 512:(qg + 1) * 512], qps,
                        mybir.ActivationFunctionType.Copy,
                    )

        # ---- attention slots 0-7, software-pipelined AV ----
        with ExitStack() as ph_a:
            ps_tr = ph_a.enter_context(tc.tile_pool(name="pstr", bufs=2, space="PSUM"))
            ps_o = ph_a.enter_context(tc.tile_pool(name="pso", bufs=1, space="PSUM"))
            p_pool = ph_a.enter_context(tc.tile_pool(name="pp", bufs=2))
            pt_pool = ph_a.enter_context(tc.tile_pool(name="ptp", bufs=2))
            sc_pool = ph_a.enter_context(tc.tile_pool(name="scp", bufs=2))
            outp = ph_a.enter_context(tc.tile_pool(name="outp", bufs=2))

            def emit_av(i, W, P_sb, recip):
                npr = W // 128               # key tiles per region (= i+1)
                stride = 512 * ((W + 511) // 512)
                L = 2 * npr
                O_ps = ps_o.tile([128, D], F32, tag="O", name=f"O{i}")
                for m in range(L):
                    r, j = divmod(m, npr)
                    g = r * 8 + j            # gather-layout V tile
                    pc = r * stride + j * 128
                    ptps = ps_tr.tile([128, 128], BF16, tag="tr", name=f"tp{i}_{m}")
                    nc.tensor.transpose(ptps, P_sb[:, pc:pc + 128], ident)
                    pt_sb = pt_pool.tile([128, 128], BF16, tag="pts", name=f"pt{i}_{m}")
                    nc.vector.tensor_copy(pt_sb, ptps)
                    for hh in range(2):
                        nc.tensor.matmul(
                            O_ps[:, hh * 512:(hh + 1) * 512], pt_sb,
                            Vt[:, g, hh * 512:(hh + 1) * 512],
                            start=(m == 0), stop=(m == L - 1),
                        )
                out_sb = outp.tile([128, D], F32, tag="osb", name=f"ou{i}")
                nc.vector.tensor_scalar_mul(out_sb, O_ps, recip)
                nc.sync.dma_start(out=out_q[i][:, :], in_=out_sb)

            def do_slot(i, ps_pool, s_alloc, prev):
                W = (i + 1) * 128            # per-region score width
                # 512-aligned region stride: every PSUM bank gets exactly one
                # start=True writer (two starts on one bank corrupt the first)
                stride = 512 * ((W + 511) // 512)
                S_ps = ps_pool.tile([128, s_alloc], F32, tag="S", name=f"S{i}")
                for e in range(8):
                    for r in range(2):
                        for off in range(0, W, 512):
                            w = min(512, W - off)
                            nc.tensor.matmul(
                                S_ps[:, r * stride + off: r * stride + off + w],
                                QT[:, e, i * 128:(i + 1) * 128],
                                KT[:, e, r * 1024 + off: r * 1024 + off + w],
                                start=(e == 0), stop=(e == 7),
                            )
                # causal masks on the tail tile of each region
                nc.vector.tensor_add(
                    S_ps[:, W - 128:W], S_ps[:, W - 128:W], mask_sb[:, 0:128]
                )
                nc.vector.tensor_add(
                    S_ps[:, stride + W - 128:stride + W],
                    S_ps[:, stride + W - 128:stride + W],
                    mask_sb[:, 128:256],
                )
                # |scores|/32 bounded (<~11) -> exp without max-subtraction
                P_sb = p_pool.tile([128, 2 * stride], BF16, tag="P", name=f"P{i}")
                stats = sc_pool.tile([128, 4], F32, tag="stats", name=f"st{i}")
                rs = [stats[:, 0:1], stats[:, 1:2]]
                for r in range(2):
                    nc.scalar.activation(
                        P_sb[:, r * stride:r * stride + W],
                        S_ps[:, r * stride:r * stride + W],
                        mybir.ActivationFunctionType.Exp,
                        bias=0.0, scale=SCALE, accum_out=rs[r],
                    )
                rowsum = stats[:, 2:3]
                nc.vector.tensor_add(rowsum, rs[0], rs[1])
                recip = stats[:, 3:4]
                nc.vector.reciprocal(recip, rowsum)
                if prev is not None:
                    emit_av(*prev)
                return (i, W, P_sb, recip)

            prev = None
            with tc.tile_pool(name="ps_sA", bufs=2, space="PSUM") as ps_sA:
                for i in range(4):
                    prev = do_slot(i, ps_sA, 1024, prev)
            with tc.tile_pool(name="ps_sB", bufs=1, space="PSUM") as ps_sB:
                for i in range(4, N_OWN):
                    prev = do_slot(i, ps_sB, 2048, prev)
                emit_av(*prev)

    nc.compile()
    return nc


def _masks():
    q = np.arange(128)[:, None]
    k = np.arange(128)[None, :]
    tril_add = np.where(k <= q, 0.0, NEG).astype(np.float32)
    m0 = np.concatenate([tril_add, np.full((128, 128), NEG, np.float32)], axis=1)
    m1 = np.concatenate([np.zeros((128, 128), np.float32), tril_add], axis=1)
    return m0, m1


def kernel(x, Wq, Wk, Wv):
    global LAST_EXEC_NS
    x = np.ascontiguousarray(np.asarray(x, dtype=np.float32))
    Wq = np.ascontiguousarray(np.asarray(Wq, dtype=np.float32))
    Wk = np.ascontiguousarray(np.asarray(Wk, dtype=np.float32))
    Wv = np.ascontiguousarray(np.asarray(Wv, dtype=np.float32))

    if "nc" not in _NC_CACHE:
        _NC_CACHE["nc"] = _build_nc()
    nc = _NC_CACHE["nc"]

    # host pre-transpose: x[b] (N, D) -> (tile, p=d%128, dchunk, token)
    # element (t, p, c, q) = x[b, t*128+q, c*128+p]
    xt_all = np.ascontiguousarray(
        x.reshape(B, 16, 128, 8, 128).transpose(0, 1, 4, 3, 2)
    )  # [B, tile, p, c, q]

    wq_r = np.ascontiguousarray(Wq.reshape(8, 128, 8, 128).transpose(2, 1, 0, 3))
    wk_r = np.ascontiguousarray(Wk.reshape(8, 128, 8, 128).transpose(2, 1, 0, 3))
    wv_r = np.ascontiguousarray(Wv.reshape(8, 128, 2, 512).transpose(2, 1, 0, 3))

    m0, m1 = _masks()
    in_maps = []
    for c in range(N_CORES):
        b, par = divmod(c, 2)
        in_maps.append({
            "x_own": np.ascontiguousarray(xt_all[b, par::2]),
            "wq": wq_r, "wk": wk_r, "wv": wv_r,
            "mask": m1 if par else m0,
        })

    res = run_bass_kernel_spmd(nc, in_maps, list(range(N_CORES)), trace=TRACE)
    LAST_EXEC_NS = res.exec_time_ns

    out = np.empty((B, N, D), dtype=np.float32)
    for c in range(N_CORES):
        b, par = divmod(c, 2)
        oq = res.results[c]["out_q"]
        for i in range(N_OWN):
            g = 2 * i + par
            out[b, g * 128:(g + 1) * 128, :] = oq[i]
    return out
